# revision 31
# baseline (speedup 1.0000x reference)
"""Cross-attention Trainium2 kernel (nn_CrossAttention_24575802868332).

Sharding: 8 cores; core c handles batch b = c//4 and query rows
r = (c%4)*1024 .. +1024.  Embarrassingly parallel, no collectives.
Host pre-transposes x and context slices and converts to bf16.

bf16 datapath (PSUM accumulation fp32): enables Fast Weight Load and
separate LDWEIGHTS that the PE reorder window can hide, which fp32r
self-loading matmuls cannot.

Schedule: ctx/x DMAs all prefetched up front; P1 (q proj) runs in two
4-bank rounds through the shared o_ps pool, interleaved after block-0
K/V projections so the PE never waits on the first ctx DMA.  The last
m-block runs qc-outer and the normalization + output projection are
emitted per q-chunk half, hiding most of the P3/P4 serial tail behind
the other half's attention slabs.

Per-core on-device computation:
  P1: q^T = Wq^T @ x^T                       qt_sb [128, 4, 1024] bf16
  P2: per m-block of 512 (8 blocks):
      k^T = Wk^T @ ctx^T block               kt [128, 4, 512] bf16
      v   = ctx^T.T @ Wv block, + ones col   vt [128, 4, 8, 65] bf16
      per (q-chunk, head-pair, m-subtile):
        S^T pair (row-packed K=64 matmuls) -> psum slab [128, 1024] f32
        P = exp(0.125 * S^T) on ScalarE    -> sbuf bf16 slab
        O_aug[65,512] += v_aug.T @ P        (row 64 = softmax denom l)
      flush O_aug psums -> acc_o [65, 16, 512] f32 (DVE copy/add)
  P3 (per qc half): l -> 8 partitions via SBUF DMA, reciprocal, K=1
      ones outer-product broadcast, normalized O written as bf16
      ko_pre, repacked into pair-packed ko_sb [128, 8, 512] bf16
  P4 (per qc half): out = (O/l) @ Wo + bo, DMA out f32
"""

import os
import sys

sys.path.insert(0, "/opt/trn_rl_repo")

from contextlib import ExitStack

import ml_dtypes
import numpy as np

import concourse.bass as bass
import concourse.tile as tile
from concourse import bacc, mybir

F32 = mybir.dt.float32
F32R = mybir.dt.float32r
BF16 = mybir.dt.bfloat16
AF = mybir.ActivationFunctionType

# Problem constants (hardcoded per contract)
B, N, M = 2, 4096, 4096
DQ, DC, INNER = 1024, 768, 512
H, D = 8, 64
NCORES = 8
NQ = N * B // NCORES  # 1024 query rows per core
QC = 2  # q chunks of 512
MBLK = 512  # m block size
NBLK = M // MBLK  # 8
HP = H // 2  # 4 head pairs
KQ = DQ // 128  # 8 k-chunks for q proj
KC = DC // 128  # 6 k-chunks for k/v proj
MS = MBLK // 128  # 4 m-subtiles per block


def build_nc():
    nc = bacc.Bacc(
        "TRN2",
        target_bir_lowering=False,
        debug=False,
        enable_asserts=False,
        num_devices=NCORES,
    )
    xT = nc.dram_tensor("xT", [DQ, NQ], BF16, kind="ExternalInput").ap()
    ctxT = nc.dram_tensor("ctxT", [DC, M], BF16, kind="ExternalInput").ap()
    wq = nc.dram_tensor("wq", [DQ, INNER], BF16, kind="ExternalInput").ap()
    wk = nc.dram_tensor("wk", [DC, INNER], BF16, kind="ExternalInput").ap()
    wv = nc.dram_tensor("wv", [DC, INNER], BF16, kind="ExternalInput").ap()
    wo = nc.dram_tensor("wo", [INNER, DQ], BF16, kind="ExternalInput").ap()
    bo = nc.dram_tensor("bo", [1, DQ], BF16, kind="ExternalInput").ap()
    ones_r = nc.dram_tensor("ones_r", [1, 128], F32R, kind="ExternalInput").ap()
    ones_b = nc.dram_tensor("ones_b", [1, 128], BF16, kind="ExternalInput").ap()
    out = nc.dram_tensor("out", [NQ, DQ], F32, kind="ExternalOutput").ap()

    with tile.TileContext(nc) as tc:
        _emit(nc, tc, xT, ctxT, wq, wk, wv, wo, bo, ones_r, ones_b, out)
    nc.compile()
    return nc


def _emit(nc, tc, xT, ctxT, wq, wk, wv, wo, bo, ones_r, ones_b, out):
    with ExitStack() as ctx:
        consts = ctx.enter_context(tc.tile_pool(name="consts", bufs=1))
        # ---- pools ----
        acc = ctx.enter_context(tc.tile_pool(name="acc", bufs=1))
        ctx_pool = ctx.enter_context(tc.tile_pool(name="ctx", bufs=3))
        kt_pool = ctx.enter_context(tc.tile_pool(name="kt", bufs=2))
        v_pool = ctx.enter_context(tc.tile_pool(name="v", bufs=2))
        p_pool = ctx.enter_context(tc.tile_pool(name="p", bufs=4))
        xt_pool = ctx.enter_context(tc.tile_pool(name="xt", bufs=KQ))
        norm = ctx.enter_context(tc.tile_pool(name="norm", bufs=1))
        out_pool = ctx.enter_context(tc.tile_pool(name="outp", bufs=2))
        bc_pool = ctx.enter_context(tc.tile_pool(name="bc", bufs=2))
        dram = ctx.enter_context(tc.tile_pool(name="dram", bufs=1, space="DRAM"))
        s_ps = ctx.enter_context(tc.tile_pool(name="sps", bufs=2, space="PSUM"))
        o_ps = ctx.enter_context(tc.tile_pool(name="ops", bufs=4, space="PSUM"))

        ctxTr = ctxT.rearrange("(k p) m -> p k m", p=128)
        xTr = xT.rearrange("(k p) q -> p k q", p=128)

        # ---- DMAs in criticality order: wq + x chunks feed the first
        # matmuls, then ctx blocks 0/1, then the remaining weights ----
        wq_sb = consts.tile([128, KQ, INNER], BF16, tag="wq")
        nc.sync.dma_start(out=wq_sb, in_=wq.rearrange("(k p) n -> p k n", p=128))
        xts = []
        for kc in range(KQ):
            xt = xt_pool.tile([128, NQ], BF16, tag="xt", name=f"xt{kc}")
            nc.sync.dma_start(out=xt, in_=xTr[:, kc, :])
            xts.append(xt)
        pref = {}
        for blk in (0, 1):
            cxp = ctx_pool.tile(
                [128, KC, MBLK], BF16, tag="cx", name=f"cx{blk}"
            )
            m0 = blk * MBLK
            nc.sync.dma_start(out=cxp, in_=ctxTr[:, :, m0 : m0 + MBLK])
            pref[blk] = cxp
        wk_sb = consts.tile([128, KC, INNER], BF16, tag="wk")
        nc.sync.dma_start(out=wk_sb, in_=wk.rearrange("(k p) n -> p k n", p=128))
        wv_sb = consts.tile([128, KC, INNER], BF16, tag="wv")
        nc.sync.dma_start(out=wv_sb, in_=wv.rearrange("(k p) n -> p k n", p=128))
        ones_bf = consts.tile([1, 128], BF16, tag="ones_bf")
        nc.sync.dma_start(out=ones_bf, in_=ones_b)
        # ones replicated to all 128 partitions (v_aug ones column source)
        ones_col = consts.tile([128, MS * H], BF16, tag="ones_col")
        ones_bcast = bass.AP(
            tensor=ones_b.tensor, offset=0, ap=[[0, 128], [1, MS * H]]
        )
        nc.gpsimd.dma_start(out=ones_col, in_=ones_bcast)
        ones_row = consts.tile([1, 128], F32R, tag="ones_row")
        nc.sync.dma_start(out=ones_row, in_=ones_r)
        # bias replicated to all 128 partitions: folded into the P4 psum
        # copy as a DVE add, replacing 16 K=1 bias matmuls on the PE
        bo_bc = consts.tile([128, DQ], BF16, tag="bo_bc")
        bo_bcast = bass.AP(tensor=bo.tensor, offset=0, ap=[[0, 128], [1, DQ]])
        nc.gpsimd.dma_start(out=bo_bc, in_=bo_bcast)
        wo_sb = consts.tile([128, INNER // 128, DQ], BF16, tag="wo")
        nc.sync.dma_start(out=wo_sb, in_=wo.rearrange("(k p) n -> p k n", p=128))

        # persistent accumulators
        # acc_o[d(0:64)+l(64), slot j = hp*4 + parity*2 + qc, q 512]
        acc_o = acc.tile([65, 16, 512], F32, tag="acc_o")
        qt_sb = acc.tile([128, HP, NQ], BF16, tag="qt")  # q^T [inner, q]

        # normalization tiles
        ko_pre = norm.tile([64, 16, 512], BF16, tag="ko_pre")
        ko_sb = norm.tile([128, H, 512], BF16, tag="ko")

        def make_block_thunks(blk):
            """Per-block projection work as single-instruction thunks,
            for sprinkling among the previous block's attention slabs."""
            st = {}
            th = []

            def t_dma(blk=blk):
                if blk in pref:
                    cx = pref[blk]
                else:
                    cx = ctx_pool.tile(
                        [128, KC, MBLK], BF16, tag="cx", name=f"cx{blk}"
                    )
                    m0 = blk * MBLK
                    nc.sync.dma_start(out=cx, in_=ctxTr[:, :, m0 : m0 + MBLK])
                st["cx"] = cx
                st["kt"] = kt_pool.tile(
                    [128, HP, MBLK], BF16, tag="kt", name=f"kt{blk}"
                )
                vt = v_pool.tile(
                    [128, MS, H, 65], BF16, tag="vt", name=f"vt{blk}"
                )
                st["vt"] = vt
                nc.vector.tensor_copy(
                    vt[:, :, :, 64:65],
                    ones_col[:].rearrange("p (a h o) -> p a h o", a=MS, h=H),
                )

            th.append(t_dma)
            for it in range(HP):
                for kc in range(KC):
                    def t_kmm(it=it, kc=kc, blk=blk):
                        if kc == 0:
                            st[f"kp{it}"] = o_ps.tile(
                                [128, 512], F32, tag="ops",
                                name=f"kp{blk}_{it}",
                            )
                        nc.tensor.matmul(
                            st[f"kp{it}"],
                            wk_sb[:, kc, it * 128 : (it + 1) * 128],
                            st["cx"][:, kc, :],
                            start=(kc == 0),
                            stop=(kc == KC - 1),
                        )
                    th.append(t_kmm)

                def t_kev(it=it):
                    nc.vector.tensor_copy(st["kt"][:, it, :], st[f"kp{it}"])

                th.append(t_kev)
            for ms in range(MS):
                for kc in range(KC):
                    def t_vmm(ms=ms, kc=kc, blk=blk):
                        if kc == 0:
                            st[f"vp{ms}"] = o_ps.tile(
                                [128, 512], F32, tag="ops",
                                name=f"vp{blk}_{ms}",
                            )
                        nc.tensor.matmul(
                            st[f"vp{ms}"],
                            st["cx"][:, kc, ms * 128 : (ms + 1) * 128],
                            wv_sb[:, kc, :],
                            start=(kc == 0),
                            stop=(kc == KC - 1),
                        )
                    th.append(t_vmm)

                def t_vev(ms=ms):
                    nc.vector.tensor_copy(
                        st["vt"][:, ms, :, 0:64],
                        st[f"vp{ms}"][:].rearrange("p (h d) -> p h d", h=H),
                    )

                th.append(t_vev)
            return st, th

        def p1_round(rnd):
            """Half of the q projection: head-pairs [2*rnd, 2*rnd+1]."""
            combos = [
                (it, qc)
                for it in (2 * rnd, 2 * rnd + 1)
                for qc in range(QC)
            ]
            accs = [
                o_ps.tile([128, 512], F32, tag="ops", name=f"q{rnd}_{i}")
                for i in range(len(combos))
            ]
            for kc in range(KQ):
                for i, (it, qc) in enumerate(combos):
                    nc.tensor.matmul(
                        accs[i],
                        wq_sb[:, kc, it * 128 : (it + 1) * 128],
                        xts[kc][:, qc * 512 : (qc + 1) * 512],
                        start=(kc == 0),
                        stop=(kc == KQ - 1),
                    )
            for i, (it, qc) in enumerate(combos):
                nc.vector.tensor_copy(
                    qt_sb[:, it, qc * 512 : (qc + 1) * 512], accs[i]
                )

        def p34_thunks(qc):
            """Normalize + repack + out-project one q-chunk's 8 slots, as
            single-step thunks so the PE pieces can be sprinkled among the
            other q-chunk's slabs (emitting them as a block would head-of-
            line-block the in-order PE queue behind the reciprocal chain)."""
            st = {}
            th = []

            def t_recip(qc=qc):
                with nc.allow_low_precision(reason="1/l in fp32r is fine"):
                    recip8 = norm.tile(
                        [8, 512], F32R, tag="recip8", name=f"recip8_{qc}"
                    )
                    nc.sync.dma_start(
                        out=recip8,
                        in_=acc_o[64:65, qc : 16 : 2, :].bitcast(F32R),
                    )
                    nc.vector.reciprocal(recip8[:], recip8[:])
                    # bounce 1/l through DRAM so it can be partition-
                    # broadcast by DMA, replacing 8 K=1 matmuls on the PE.
                    # The raw-AP read below is ordered after this write by
                    # the sync DMA queue's FIFO.
                    rd = dram.tile([8, 512], F32R, name=f"recd_{qc}")
                    nc.sync.dma_start(out=rd, in_=recip8[:])
                    st["rd"] = rd

            th.append(t_recip)
            for i in range(8):
                def t_norm(i=i, qc=qc):
                    j = 2 * i + qc
                    bc = bc_pool.tile(
                        [64, 512], F32R, tag="bc", name=f"bc{j}"
                    )
                    rdap = st["rd"][:]
                    nc.sync.dma_start(
                        out=bc,
                        in_=bass.AP(
                            tensor=rdap.tensor,
                            offset=rdap.offset + i * 512,
                            ap=[[0, 64], [1, 512]],
                        ),
                    )
                    with nc.allow_low_precision(reason="O/l to bf16"):
                        nc.vector.tensor_mul(
                            ko_pre[:, j, :],
                            acc_o[0:64, j, :],
                            bc[:].bitcast(F32),
                        )

                th.append(t_norm)
            for hp in range(HP):
                def t_repack(hp=hp, qc=qc):
                    j2 = hp * 2 + qc
                    nc.sync.dma_start(
                        out=ko_sb[0:64, j2, :], in_=ko_pre[:, hp * 4 + qc, :]
                    )
                    nc.sync.dma_start(
                        out=ko_sb[64:128, j2, :],
                        in_=ko_pre[:, hp * 4 + 2 + qc, :],
                    )

                th.append(t_repack)
            for qt_i in range(qc * 4, qc * 4 + 4):
                for nck in range(DQ // 512):
                    def t_oproj(qt_i=qt_i, nck=nck, qc=qc):
                        ql = qt_i % 4
                        if nck == 0:
                            st[f"ob{qt_i}"] = out_pool.tile(
                                [128, DQ], F32, tag="outp", name=f"ob{qt_i}"
                            )
                        ob = st[f"ob{qt_i}"]
                        pp = o_ps.tile(
                            [128, 512], F32, tag="ops", name=f"pp{qt_i}_{nck}"
                        )
                        for hp in range(HP):
                            nc.tensor.matmul(
                                pp,
                                ko_sb[:, hp * 2 + qc, ql * 128 : (ql + 1) * 128],
                                wo_sb[:, hp, nck * 512 : (nck + 1) * 512],
                                start=(hp == 0),
                                stop=(hp == HP - 1),
                            )
                        nc.vector.tensor_add(
                            ob[:, nck * 512 : (nck + 1) * 512],
                            pp,
                            bo_bc[:, nck * 512 : (nck + 1) * 512],
                        )
                        if nck == DQ // 512 - 1:
                            nc.sync.dma_start(
                                out=out[qt_i * 128 : (qt_i + 1) * 128, :],
                                in_=ob,
                            )

                    th.append(t_oproj)
            return th

        # ---- prologue: P1 round A, block-0 projections, P1 round B ----
        p1_round(0)
        cur_st, th0 = make_block_thunks(0)
        for t in th0:
            t()
        p1_round(1)

        # ---- P2: m-block loop ----
        for blk in range(NBLK):
            kt = cur_st["kt"]
            vt = cur_st["vt"]
            if blk + 1 < NBLK:
                next_st, pend = make_block_thunks(blk + 1)
            else:
                next_st, pend = None, []
            # pop ~evenly over the 32 slab iterations
            n_slabs = HP * QC * MS
            per = (len(pend) + n_slabs - 1) // n_slabs if pend else 0

            last = blk == NBLK - 1
            for qc in range(QC):
                skip = 0
                if last and qc == 1:
                    # overlap the first q-chunk's normalization + output
                    # projection with this q-chunk's attention slabs; delay
                    # the PE pieces until the reciprocal chain has landed
                    pend = p34_thunks(0)
                    per = 3
                    skip = 4
                slab_i = 0
                for hp in range(HP):
                    ops_e = o_ps.tile(
                        [65, 512], F32, tag="ops", name=f"oe{blk}_{hp}_{qc}"
                    )
                    ops_o = o_ps.tile(
                        [65, 512], F32, tag="ops", name=f"oo{blk}_{hp}_{qc}"
                    )
                    o_emits = []
                    for mt in range(MS):
                        sl = s_ps.tile(
                            [128, 1024], F32, tag="sps",
                            name=f"sl{blk}_{hp}_{qc}_{mt}",
                        )
                        nc.tensor.matmul(
                            sl[:, 0:512],
                            kt[0:64, hp, mt * 128 : (mt + 1) * 128],
                            qt_sb[0:64, hp, qc * 512 : (qc + 1) * 512],
                            start=True,
                            stop=True,
                        )
                        nc.tensor.matmul(
                            sl[:, 512:1024],
                            kt[64:128, hp, mt * 128 : (mt + 1) * 128],
                            qt_sb[64:128, hp, qc * 512 : (qc + 1) * 512],
                            start=True,
                            stop=True,
                        )
                        psl = p_pool.tile(
                            [128, 1024], BF16, tag="p",
                            name=f"psl{blk}_{hp}_{qc}_{mt}",
                        )
                        nc.scalar.activation(psl, sl, AF.Exp, scale=0.125)

                        def o_pair(mt=mt, psl=psl, ops_e=ops_e, ops_o=ops_o,
                                   hp=hp, vt=vt):
                            nc.tensor.matmul(
                                ops_e,
                                vt[:, mt, 2 * hp, :],
                                psl[:, 0:512],
                                start=(mt == 0),
                                stop=(mt == MS - 1),
                            )
                            nc.tensor.matmul(
                                ops_o,
                                vt[:, mt, 2 * hp + 1, :],
                                psl[:, 512:1024],
                                start=(mt == 0),
                                stop=(mt == MS - 1),
                            )

                        o_emits.append(o_pair)
                        # software pipeline: O lags S by one slab
                        if mt >= 1:
                            o_emits.pop(0)()
                        # sprinkle next block's projection work (or, on the
                        # last block, the first q-chunk's P3/P4 thunks)
                        if slab_i >= skip:
                            for _ in range(per):
                                if pend:
                                    pend.pop(0)()
                        slab_i += 1
                    while o_emits:
                        o_emits.pop(0)()
                    # flush to accumulators
                    je = hp * 4 + 0 * 2 + qc
                    jo = hp * 4 + 1 * 2 + qc
                    if blk == 0:
                        nc.vector.tensor_copy(acc_o[:, je, :], ops_e)
                        nc.vector.tensor_copy(acc_o[:, jo, :], ops_o)
                    else:
                        nc.vector.tensor_add(
                            acc_o[:, je, :], acc_o[:, je, :], ops_e
                        )
                        nc.vector.tensor_add(
                            acc_o[:, jo, :], acc_o[:, jo, :], ops_o
                        )
            for t in pend:  # any leftovers
                t()
            if last:
                # second q-chunk's normalization + output projection
                for t in p34_thunks(1):
                    t()
            if next_st is not None:
                cur_st = next_st


_NC_CACHE = None


def _get_nc():
    global _NC_CACHE
    if _NC_CACHE is None:
        _NC_CACHE = build_nc()
    return _NC_CACHE


def shard_inputs(x, context, Wq, Wk, Wv, Wo, bo):
    bf = ml_dtypes.bfloat16
    ones_r = np.ones((1, 128), np.float32)
    ones_b = np.ones((1, 128), bf)
    bo2 = np.ascontiguousarray(np.asarray(bo, np.float32).reshape(1, DQ)).astype(bf)
    Wq = np.ascontiguousarray(np.asarray(Wq, np.float32)).astype(bf)
    Wk = np.ascontiguousarray(np.asarray(Wk, np.float32)).astype(bf)
    Wv = np.ascontiguousarray(np.asarray(Wv, np.float32)).astype(bf)
    Wo = np.ascontiguousarray(np.asarray(Wo, np.float32)).astype(bf)
    maps = []
    for c in range(NCORES):
        b = c // 4
        r0 = (c % 4) * NQ
        maps.append(
            {
                "xT": np.ascontiguousarray(x[b, r0 : r0 + NQ, :].T).astype(bf),
                "ctxT": np.ascontiguousarray(context[b].T).astype(bf),
                "wq": Wq,
                "wk": Wk,
                "wv": Wv,
                "wo": Wo,
                "bo": bo2,
                "ones_r": ones_r,
                "ones_b": ones_b,
            }
        )
    return maps


def kernel(x, context, Wq, Wk, Wv, Wo, bo):
    from concourse.bass_utils import run_bass_kernel_spmd

    x = np.asarray(x, np.float32)
    context = np.asarray(context, np.float32)
    maps = shard_inputs(x, context, Wq, Wk, Wv, Wo, bo)
    nc = _get_nc()
    trace = os.environ.get("KERNEL_TRACE", "0") == "1"
    res = run_bass_kernel_spmd(
        nc, maps, core_ids=list(range(NCORES)), trace=trace
    )
    full = np.empty((B, N, DQ), np.float32)
    for c in range(NCORES):
        b = c // 4
        r0 = (c % 4) * NQ
        full[b, r0 : r0 + NQ, :] = res.results[c]["out"]
    if trace:
        kernel.last_exec_time_ns = res.exec_time_ns
    return full


# revision 32
# speedup vs baseline: 1.0200x; 1.0200x over previous
"""Cross-attention Trainium2 kernel (nn_CrossAttention_24575802868332).

Sharding: 8 cores; core c handles batch b = c//4 and query rows
r = (c%4)*1024 .. +1024.  Embarrassingly parallel, no collectives.
Host pre-transposes x and context slices and converts to bf16.

bf16 datapath (PSUM accumulation fp32): enables Fast Weight Load and
separate LDWEIGHTS that the PE reorder window can hide, which fp32r
self-loading matmuls cannot.

Schedule: ctx/x DMAs all prefetched up front; P1 (q proj) runs in two
4-bank rounds through the shared o_ps pool, interleaved after block-0
K/V projections so the PE never waits on the first ctx DMA.  The last
m-block runs qc-outer and the normalization + output projection are
emitted per q-chunk half, hiding most of the P3/P4 serial tail behind
the other half's attention slabs.

Per-core on-device computation:
  P1: q^T = Wq^T @ x^T                       qt_sb [128, 4, 1024] bf16
  P2: per m-block of 512 (8 blocks):
      k^T = Wk^T @ ctx^T block               kt [128, 4, 512] bf16
      v   = ctx^T.T @ Wv block, + ones col   vt [128, 4, 8, 65] bf16
      per (q-chunk, head-pair, m-subtile):
        S^T pair (row-packed K=64 matmuls) -> psum slab [128, 1024] f32
        P = exp(0.125 * S^T) on ScalarE    -> sbuf bf16 slab
        O_aug[65,512] += v_aug.T @ P        (row 64 = softmax denom l)
      flush O_aug psums -> acc_o [65, 16, 512] f32 (DVE copy/add)
  P3 (per qc half): l -> 8 partitions via SBUF DMA, reciprocal, K=1
      ones outer-product broadcast, normalized O written as bf16
      ko_pre, repacked into pair-packed ko_sb [128, 8, 512] bf16
  P4 (per qc half): out = (O/l) @ Wo + bo, DMA out f32
"""

import os
import sys

sys.path.insert(0, "/opt/trn_rl_repo")

from contextlib import ExitStack

import ml_dtypes
import numpy as np

import concourse.bass as bass
import concourse.tile as tile
from concourse import bacc, mybir

F32 = mybir.dt.float32
F32R = mybir.dt.float32r
BF16 = mybir.dt.bfloat16
AF = mybir.ActivationFunctionType

# Problem constants (hardcoded per contract)
B, N, M = 2, 4096, 4096
DQ, DC, INNER = 1024, 768, 512
H, D = 8, 64
NCORES = 8
NQ = N * B // NCORES  # 1024 query rows per core
QC = 2  # q chunks of 512
MBLK = 512  # m block size
NBLK = M // MBLK  # 8
HP = H // 2  # 4 head pairs
KQ = DQ // 128  # 8 k-chunks for q proj
KC = DC // 128  # 6 k-chunks for k/v proj
MS = MBLK // 128  # 4 m-subtiles per block


def build_nc():
    nc = bacc.Bacc(
        "TRN2",
        target_bir_lowering=False,
        debug=False,
        enable_asserts=False,
        num_devices=NCORES,
    )
    xT = nc.dram_tensor("xT", [DQ, NQ], BF16, kind="ExternalInput").ap()
    ctxT = nc.dram_tensor("ctxT", [DC, M], BF16, kind="ExternalInput").ap()
    wq = nc.dram_tensor("wq", [DQ, INNER], BF16, kind="ExternalInput").ap()
    wk = nc.dram_tensor("wk", [DC, INNER], BF16, kind="ExternalInput").ap()
    wv = nc.dram_tensor("wv", [DC, INNER], BF16, kind="ExternalInput").ap()
    wo = nc.dram_tensor("wo", [INNER, DQ], BF16, kind="ExternalInput").ap()
    bo = nc.dram_tensor("bo", [1, DQ], BF16, kind="ExternalInput").ap()
    ones_r = nc.dram_tensor("ones_r", [1, 128], F32R, kind="ExternalInput").ap()
    ones_b = nc.dram_tensor("ones_b", [1, 128], BF16, kind="ExternalInput").ap()
    out = nc.dram_tensor("out", [NQ, DQ], F32, kind="ExternalOutput").ap()

    with tile.TileContext(nc) as tc:
        _emit(nc, tc, xT, ctxT, wq, wk, wv, wo, bo, ones_r, ones_b, out)
    nc.compile()
    return nc


def _emit(nc, tc, xT, ctxT, wq, wk, wv, wo, bo, ones_r, ones_b, out):
    with ExitStack() as ctx:
        consts = ctx.enter_context(tc.tile_pool(name="consts", bufs=1))
        # ---- pools ----
        acc = ctx.enter_context(tc.tile_pool(name="acc", bufs=1))
        ctx_pool = ctx.enter_context(tc.tile_pool(name="ctx", bufs=3))
        kt_pool = ctx.enter_context(tc.tile_pool(name="kt", bufs=2))
        v_pool = ctx.enter_context(tc.tile_pool(name="v", bufs=2))
        p_pool = ctx.enter_context(tc.tile_pool(name="p", bufs=4))
        xt_pool = ctx.enter_context(tc.tile_pool(name="xt", bufs=KQ))
        norm = ctx.enter_context(tc.tile_pool(name="norm", bufs=1))
        out_pool = ctx.enter_context(tc.tile_pool(name="outp", bufs=2))
        s_ps = ctx.enter_context(tc.tile_pool(name="sps", bufs=2, space="PSUM"))
        o_ps = ctx.enter_context(tc.tile_pool(name="ops", bufs=4, space="PSUM"))

        ctxTr = ctxT.rearrange("(k p) m -> p k m", p=128)
        xTr = xT.rearrange("(k p) q -> p k q", p=128)

        # ---- DMAs in criticality order: wq + x chunks feed the first
        # matmuls, then ctx blocks 0/1, then the remaining weights ----
        wq_sb = consts.tile([128, KQ, INNER], BF16, tag="wq")
        nc.sync.dma_start(out=wq_sb, in_=wq.rearrange("(k p) n -> p k n", p=128))
        xts = []
        for kc in range(KQ):
            xt = xt_pool.tile([128, NQ], BF16, tag="xt", name=f"xt{kc}")
            nc.sync.dma_start(out=xt, in_=xTr[:, kc, :])
            xts.append(xt)
        pref = {}
        for blk in (0, 1):
            cxp = ctx_pool.tile(
                [128, KC, MBLK], BF16, tag="cx", name=f"cx{blk}"
            )
            m0 = blk * MBLK
            nc.sync.dma_start(out=cxp, in_=ctxTr[:, :, m0 : m0 + MBLK])
            pref[blk] = cxp
        wk_sb = consts.tile([128, KC, INNER], BF16, tag="wk")
        nc.sync.dma_start(out=wk_sb, in_=wk.rearrange("(k p) n -> p k n", p=128))
        wv_sb = consts.tile([128, KC, INNER], BF16, tag="wv")
        nc.sync.dma_start(out=wv_sb, in_=wv.rearrange("(k p) n -> p k n", p=128))
        ones_bf = consts.tile([1, 128], BF16, tag="ones_bf")
        nc.sync.dma_start(out=ones_bf, in_=ones_b)
        # ones replicated to all 128 partitions (v_aug ones column source)
        ones_col = consts.tile([128, MS * H], BF16, tag="ones_col")
        ones_bcast = bass.AP(
            tensor=ones_b.tensor, offset=0, ap=[[0, 128], [1, MS * H]]
        )
        nc.gpsimd.dma_start(out=ones_col, in_=ones_bcast)
        ones_row = consts.tile([1, 128], F32R, tag="ones_row")
        nc.sync.dma_start(out=ones_row, in_=ones_r)
        bo_sb = consts.tile([1, DQ], BF16, tag="bo")
        nc.sync.dma_start(out=bo_sb, in_=bo)
        wo_sb = consts.tile([128, INNER // 128, DQ], BF16, tag="wo")
        nc.sync.dma_start(out=wo_sb, in_=wo.rearrange("(k p) n -> p k n", p=128))

        # persistent accumulators
        # acc_o[d(0:64)+l(64), slot j = hp*4 + parity*2 + qc, q 512]
        acc_o = acc.tile([65, 16, 512], F32, tag="acc_o")
        qt_sb = acc.tile([128, HP, NQ], BF16, tag="qt")  # q^T [inner, q]

        # normalization tiles
        ko_pre = norm.tile([64, 16, 512], BF16, tag="ko_pre")
        ko_sb = norm.tile([128, H, 512], BF16, tag="ko")

        def make_block_thunks(blk):
            """Per-block projection work as single-instruction thunks,
            for sprinkling among the previous block's attention slabs."""
            st = {}
            th = []

            def t_dma(blk=blk):
                if blk in pref:
                    cx = pref[blk]
                else:
                    cx = ctx_pool.tile(
                        [128, KC, MBLK], BF16, tag="cx", name=f"cx{blk}"
                    )
                    m0 = blk * MBLK
                    nc.sync.dma_start(out=cx, in_=ctxTr[:, :, m0 : m0 + MBLK])
                st["cx"] = cx
                st["kt"] = kt_pool.tile(
                    [128, HP, MBLK], BF16, tag="kt", name=f"kt{blk}"
                )
                vt = v_pool.tile(
                    [128, MS, H, 65], BF16, tag="vt", name=f"vt{blk}"
                )
                st["vt"] = vt
                nc.vector.tensor_copy(
                    vt[:, :, :, 64:65],
                    ones_col[:].rearrange("p (a h o) -> p a h o", a=MS, h=H),
                )

            th.append(t_dma)
            for it in range(HP):
                for kc in range(KC):
                    def t_kmm(it=it, kc=kc, blk=blk):
                        if kc == 0:
                            st[f"kp{it}"] = o_ps.tile(
                                [128, 512], F32, tag="ops",
                                name=f"kp{blk}_{it}",
                            )
                        nc.tensor.matmul(
                            st[f"kp{it}"],
                            wk_sb[:, kc, it * 128 : (it + 1) * 128],
                            st["cx"][:, kc, :],
                            start=(kc == 0),
                            stop=(kc == KC - 1),
                        )
                    th.append(t_kmm)

                def t_kev(it=it):
                    nc.vector.tensor_copy(st["kt"][:, it, :], st[f"kp{it}"])

                th.append(t_kev)
            for ms in range(MS):
                for kc in range(KC):
                    def t_vmm(ms=ms, kc=kc, blk=blk):
                        if kc == 0:
                            st[f"vp{ms}"] = o_ps.tile(
                                [128, 512], F32, tag="ops",
                                name=f"vp{blk}_{ms}",
                            )
                        nc.tensor.matmul(
                            st[f"vp{ms}"],
                            st["cx"][:, kc, ms * 128 : (ms + 1) * 128],
                            wv_sb[:, kc, :],
                            start=(kc == 0),
                            stop=(kc == KC - 1),
                        )
                    th.append(t_vmm)

                def t_vev(ms=ms):
                    nc.vector.tensor_copy(
                        st["vt"][:, ms, :, 0:64],
                        st[f"vp{ms}"][:].rearrange("p (h d) -> p h d", h=H),
                    )

                th.append(t_vev)
            return st, th

        def p1_round(rnd):
            """Half of the q projection: head-pairs [2*rnd, 2*rnd+1]."""
            combos = [
                (it, qc)
                for it in (2 * rnd, 2 * rnd + 1)
                for qc in range(QC)
            ]
            accs = [
                o_ps.tile([128, 512], F32, tag="ops", name=f"q{rnd}_{i}")
                for i in range(len(combos))
            ]
            for kc in range(KQ):
                for i, (it, qc) in enumerate(combos):
                    nc.tensor.matmul(
                        accs[i],
                        wq_sb[:, kc, it * 128 : (it + 1) * 128],
                        xts[kc][:, qc * 512 : (qc + 1) * 512],
                        start=(kc == 0),
                        stop=(kc == KQ - 1),
                    )
            for i, (it, qc) in enumerate(combos):
                nc.vector.tensor_copy(
                    qt_sb[:, it, qc * 512 : (qc + 1) * 512], accs[i]
                )

        def p34_thunks(qc):
            """Normalize + repack + out-project one q-chunk's 8 slots, as
            single-step thunks so the PE pieces can be sprinkled among the
            other q-chunk's slabs (emitting them as a block would head-of-
            line-block the in-order PE queue behind the reciprocal chain)."""
            st = {}
            th = []

            def t_recip(qc=qc):
                with nc.allow_low_precision(reason="1/l in fp32r is fine"):
                    recip8 = norm.tile(
                        [8, 512], F32R, tag="recip8", name=f"recip8_{qc}"
                    )
                    nc.sync.dma_start(
                        out=recip8,
                        in_=acc_o[64:65, qc : 16 : 2, :].bitcast(F32R),
                    )
                    nc.vector.reciprocal(recip8[:], recip8[:])
                    recip = norm.tile(
                        [1, 8, 512], F32R, tag="recip", name=f"recip_{qc}"
                    )
                    nc.sync.dma_start(out=recip, in_=recip8[:, :])
                    st["recip"] = recip

            th.append(t_recip)
            for i in range(8):
                def t_norm(i=i, qc=qc):
                    j = 2 * i + qc
                    bp = o_ps.tile([64, 512], F32, tag="ops", name=f"bp{j}")
                    nc.tensor.matmul(
                        bp,
                        ones_row[0:1, 0:64],
                        st["recip"][:, i, :],
                        start=True,
                        stop=True,
                    )
                    with nc.allow_low_precision(reason="O/l to bf16"):
                        nc.vector.tensor_mul(
                            ko_pre[:, j, :], acc_o[0:64, j, :], bp
                        )

                th.append(t_norm)
            for hp in range(HP):
                def t_repack(hp=hp, qc=qc):
                    j2 = hp * 2 + qc
                    nc.sync.dma_start(
                        out=ko_sb[0:64, j2, :], in_=ko_pre[:, hp * 4 + qc, :]
                    )
                    nc.sync.dma_start(
                        out=ko_sb[64:128, j2, :],
                        in_=ko_pre[:, hp * 4 + 2 + qc, :],
                    )

                th.append(t_repack)
            for qt_i in range(qc * 4, qc * 4 + 4):
                for nck in range(DQ // 512):
                    def t_oproj(qt_i=qt_i, nck=nck, qc=qc):
                        ql = qt_i % 4
                        if nck == 0:
                            st[f"ob{qt_i}"] = out_pool.tile(
                                [128, DQ], F32, tag="outp", name=f"ob{qt_i}"
                            )
                        ob = st[f"ob{qt_i}"]
                        pp = o_ps.tile(
                            [128, 512], F32, tag="ops", name=f"pp{qt_i}_{nck}"
                        )
                        for hp in range(HP):
                            nc.tensor.matmul(
                                pp,
                                ko_sb[:, hp * 2 + qc, ql * 128 : (ql + 1) * 128],
                                wo_sb[:, hp, nck * 512 : (nck + 1) * 512],
                                start=(hp == 0),
                                stop=False,
                            )
                        nc.tensor.matmul(
                            pp,
                            ones_bf[0:1, :],
                            bo_sb[0:1, nck * 512 : (nck + 1) * 512],
                            start=False,
                            stop=True,
                        )
                        nc.vector.tensor_copy(
                            ob[:, nck * 512 : (nck + 1) * 512], pp
                        )
                        if nck == DQ // 512 - 1:
                            nc.sync.dma_start(
                                out=out[qt_i * 128 : (qt_i + 1) * 128, :],
                                in_=ob,
                            )

                    th.append(t_oproj)
            return th

        # ---- prologue: P1 round A, block-0 projections, P1 round B ----
        p1_round(0)
        cur_st, th0 = make_block_thunks(0)
        for t in th0:
            t()
        p1_round(1)

        # ---- P2: m-block loop ----
        for blk in range(NBLK):
            kt = cur_st["kt"]
            vt = cur_st["vt"]
            if blk + 1 < NBLK:
                next_st, pend = make_block_thunks(blk + 1)
            else:
                next_st, pend = None, []
            # pop ~evenly over the 32 slab iterations
            n_slabs = HP * QC * MS
            per = (len(pend) + n_slabs - 1) // n_slabs if pend else 0

            last = blk == NBLK - 1
            for qc in range(QC):
                skip = 0
                if last and qc == 1:
                    # overlap the first q-chunk's normalization + output
                    # projection with this q-chunk's attention slabs; delay
                    # the PE pieces until the reciprocal chain has landed
                    pend = p34_thunks(0)
                    per = 3
                    skip = 4
                slab_i = 0
                for hp in range(HP):
                    ops_e = o_ps.tile(
                        [65, 512], F32, tag="ops", name=f"oe{blk}_{hp}_{qc}"
                    )
                    ops_o = o_ps.tile(
                        [65, 512], F32, tag="ops", name=f"oo{blk}_{hp}_{qc}"
                    )
                    o_emits = []
                    for mt in range(MS):
                        sl = s_ps.tile(
                            [128, 1024], F32, tag="sps",
                            name=f"sl{blk}_{hp}_{qc}_{mt}",
                        )
                        nc.tensor.matmul(
                            sl[:, 0:512],
                            kt[0:64, hp, mt * 128 : (mt + 1) * 128],
                            qt_sb[0:64, hp, qc * 512 : (qc + 1) * 512],
                            start=True,
                            stop=True,
                        )
                        nc.tensor.matmul(
                            sl[:, 512:1024],
                            kt[64:128, hp, mt * 128 : (mt + 1) * 128],
                            qt_sb[64:128, hp, qc * 512 : (qc + 1) * 512],
                            start=True,
                            stop=True,
                        )
                        psl = p_pool.tile(
                            [128, 1024], BF16, tag="p",
                            name=f"psl{blk}_{hp}_{qc}_{mt}",
                        )
                        nc.scalar.activation(psl, sl, AF.Exp, scale=0.125)

                        def o_pair(mt=mt, psl=psl, ops_e=ops_e, ops_o=ops_o,
                                   hp=hp, vt=vt):
                            nc.tensor.matmul(
                                ops_e,
                                vt[:, mt, 2 * hp, :],
                                psl[:, 0:512],
                                start=(mt == 0),
                                stop=(mt == MS - 1),
                            )
                            nc.tensor.matmul(
                                ops_o,
                                vt[:, mt, 2 * hp + 1, :],
                                psl[:, 512:1024],
                                start=(mt == 0),
                                stop=(mt == MS - 1),
                            )

                        o_emits.append(o_pair)
                        # software pipeline: O lags S by one slab
                        if mt >= 1:
                            o_emits.pop(0)()
                        # sprinkle next block's projection work (or, on the
                        # last block, the first q-chunk's P3/P4 thunks)
                        if slab_i >= skip:
                            for _ in range(per):
                                if pend:
                                    pend.pop(0)()
                        slab_i += 1
                    while o_emits:
                        o_emits.pop(0)()
                    # flush to accumulators
                    je = hp * 4 + 0 * 2 + qc
                    jo = hp * 4 + 1 * 2 + qc
                    if blk == 0:
                        nc.vector.tensor_copy(acc_o[:, je, :], ops_e)
                        nc.vector.tensor_copy(acc_o[:, jo, :], ops_o)
                    else:
                        nc.vector.tensor_add(
                            acc_o[:, je, :], acc_o[:, je, :], ops_e
                        )
                        nc.vector.tensor_add(
                            acc_o[:, jo, :], acc_o[:, jo, :], ops_o
                        )
            for t in pend:  # any leftovers
                t()
            if last:
                # second q-chunk's normalization + output projection
                for t in p34_thunks(1):
                    t()
            if next_st is not None:
                cur_st = next_st


_NC_CACHE = None


def _get_nc():
    global _NC_CACHE
    if _NC_CACHE is None:
        _NC_CACHE = build_nc()
    return _NC_CACHE


def shard_inputs(x, context, Wq, Wk, Wv, Wo, bo):
    bf = ml_dtypes.bfloat16
    ones_r = np.ones((1, 128), np.float32)
    ones_b = np.ones((1, 128), bf)
    bo2 = np.ascontiguousarray(np.asarray(bo, np.float32).reshape(1, DQ)).astype(bf)
    Wq = np.ascontiguousarray(np.asarray(Wq, np.float32)).astype(bf)
    Wk = np.ascontiguousarray(np.asarray(Wk, np.float32)).astype(bf)
    Wv = np.ascontiguousarray(np.asarray(Wv, np.float32)).astype(bf)
    Wo = np.ascontiguousarray(np.asarray(Wo, np.float32)).astype(bf)
    maps = []
    for c in range(NCORES):
        b = c // 4
        r0 = (c % 4) * NQ
        maps.append(
            {
                "xT": np.ascontiguousarray(x[b, r0 : r0 + NQ, :].T).astype(bf),
                "ctxT": np.ascontiguousarray(context[b].T).astype(bf),
                "wq": Wq,
                "wk": Wk,
                "wv": Wv,
                "wo": Wo,
                "bo": bo2,
                "ones_r": ones_r,
                "ones_b": ones_b,
            }
        )
    return maps


def kernel(x, context, Wq, Wk, Wv, Wo, bo):
    from concourse.bass_utils import run_bass_kernel_spmd

    x = np.asarray(x, np.float32)
    context = np.asarray(context, np.float32)
    maps = shard_inputs(x, context, Wq, Wk, Wv, Wo, bo)
    nc = _get_nc()
    trace = os.environ.get("KERNEL_TRACE", "0") == "1"
    res = run_bass_kernel_spmd(
        nc, maps, core_ids=list(range(NCORES)), trace=trace
    )
    full = np.empty((B, N, DQ), np.float32)
    for c in range(NCORES):
        b = c // 4
        r0 = (c % 4) * NQ
        full[b, r0 : r0 + NQ, :] = res.results[c]["out"]
    if trace:
        kernel.last_exec_time_ns = res.exec_time_ns
    return full
